# revision 15
# baseline (speedup 1.0000x reference)
"""Trainium2 kernel for nn_MiddleHeadLayer: 2-layer tanh MLP + row-dot + sigmoid.

    inner = tanh(batch @ W1.T + b1)        batch [N, 1024], W1 [4096, 1024]
    wx    = tanh(inner @ W2.T + b2)        W2 [1024, 4096]
    out   = sigmoid(sum(wx * batch, -1))   [N]

Data-parallel over 8 NeuronCores: each core handles N/8 = 2048 rows;
weights replicated, resident in SBUF as fp16 (fp16 matmuls run at full PE
rate, 1 moving column/cycle; absmax error stays ~4e-3).

Per-core dataflow, in blocks of R=512 rows:
  phase 1: innerT[dff, rows] = tanh(W1T.T @ batchT + b1) — stationary W1T
           chunks [128,128], moving batchT [128, 512], fp16 in / f32 PSUM,
           ACT applies the per-partition (d_ff) bias and writes fp16.
  phase 2: wxT[dmodel, rows] = tanh(W2 @ inner.T + b2) — stationary W2T
           chunks [128 dff, 128 dmodel], moving innerT [128, 512]. Output
           partitions are d_model, so b2 is a per-partition ACT bias.
  dot:     acc[dm_chunk, rows] = sum_h wxT_h * batchT_h on DVE (reusing the
           fp16 batchT tiles phase 1 streams); final add writes fp16 so the
           [128,1]-ones partition-reduce matmul runs in one PE pass;
           sigmoid on ACT; one contiguous 2KB output DMA per block.

DMA strategy: the Sync sequencer issues one DMA every ~580ns, so DMA
COUNT (not just bytes) is the startup constraint. All weights and
activations are host-packed so each logical group is ONE contiguous
[128, 4096] 1MB DMA (~78% DMA efficiency): 8 for W1, 8 for W2, 4 for
batchT. The 128-partition slab meaning of each 512-column span differs
(it encodes the contraction chunk), which the matmul APs slice out.
A memset-fed burst of junk matmuls warms the PE clock gate (HAM,
1.2 -> 2.4 GHz) while the first DMAs are in flight.
"""

from contextlib import ExitStack

import numpy as np
import orjson

import concourse.bass as bass
import concourse.tile as tile
from concourse import mybir
from concourse import bass_utils

D_MODEL = 1024
D_FF = 4096
N_TOTAL = 16384
N_CORES = 8
NC_ROWS = N_TOTAL // N_CORES          # 2048 rows per core
R = 512                               # row-block size
N_BLOCKS = NC_ROWS // R               # 4
K1 = D_MODEL // 128                   # 8 contraction chunks for matmul1
M1 = D_FF // 128                      # 32 d_ff chunks
MB = 4                                # m-chunks per w1 column block
NMB = M1 // MB                        # 8 w1 column blocks
H = D_MODEL // 128                    # 8 d_model chunks in phase 2
F16 = mybir.dt.float16
F32 = mybir.dt.float32
N_WARM = 96                           # HAM warm-up matmuls during DMA wait


# ---------------------------------------------------------------------------
# This walrus build rejects >2 sem waits on a single instruction, while Tile's
# wait assignment freely attaches more (e.g. the exit drain gets one wait per
# outstanding logical proc). Legalize at the BIR-JSON level: hoist excess
# waits onto EventSemaphore instructions inserted directly before the
# offending instruction on the same engine stream (identical semantics).
MAX_WAITS = 1
ESEM_WAITS = 2  # EventSemaphore instructions themselves may carry 2 waits


def _legalize_sync_waits(bir: dict) -> dict:
    ctr = 0
    for fn in bir.get("functions", []):
        for blk in fn.get("blocks", []):
            insts = blk.get("instructions")
            if not insts:
                continue
            out = []
            changed = False
            for inst in insts:
                si = inst.get("sync_info")
                ow = (si or {}).get("on_wait") or []
                limit = ESEM_WAITS if inst.get("opcode") == "EventSemaphore" else MAX_WAITS
                if len(ow) > limit:
                    changed = True
                    excess, keep = ow[:-limit], ow[-limit:]
                    for i in range(0, len(excess), ESEM_WAITS):
                        ctr += 1
                        out.append({
                            "debug": inst.get("debug"),
                            "engine": inst["engine"],
                            "ins": [],
                            "outs": [],
                            "name": f"legalwait-{ctr}",
                            "opcode": "EventSemaphore",
                            "sync_info": {
                                "on_update": [],
                                "on_wait": excess[i:i + ESEM_WAITS],
                            },
                        })
                    si["on_wait"] = keep
                out.append(inst)
            if changed:
                blk["instructions"] = out
    return bir


_orig_to_json_bytes = bass.Bass.to_json_bytes


def _patched_to_json_bytes(self) -> bytes:
    return orjson.dumps(_legalize_sync_waits(orjson.loads(_orig_to_json_bytes(self))))


bass.Bass.to_json_bytes = _patched_to_json_bytes


def build_bass(n_blocks=N_BLOCKS):
    nc = bass.Bass("TRN2", target_bir_lowering=False, debug=False)

    # w1p row-block mb: [128, 4096], cols k*512 + mo*128 + c hold
    #   W1T[k*128+p, mb*512 + mo*128 + c]
    w1p_d = nc.dram_tensor("w1p", [NMB * 128, K1 * 512], F16, kind="ExternalInput")
    # w2p row-block h: [128, 4096], cols m*128 + c hold W2T[m*128+p, h*128+c]
    w2p_d = nc.dram_tensor("w2p", [H * 128, M1 * 128], F16, kind="ExternalInput")
    b1_d = nc.dram_tensor("b1c", [128, M1], F32, kind="ExternalInput")
    b2_d = nc.dram_tensor("b2c", [128, H], F32, kind="ExternalInput")
    ones_d = nc.dram_tensor("ones", [128, 1], F16, kind="ExternalInput")
    # btp row-block b: [128, 4096], cols k*512 + r hold batch[b*512+r, k*128+p]
    btp_d = nc.dram_tensor("btp", [N_BLOCKS * 128, K1 * R], F16, kind="ExternalInput")
    out_d = nc.dram_tensor("out", [1, NC_ROWS], F32, kind="ExternalOutput")

    with tile.TileContext(nc) as tc, ExitStack() as ctx:
        wpool = ctx.enter_context(tc.tile_pool(name="weights", bufs=1))
        apool = ctx.enter_context(tc.tile_pool(name="acts", bufs=2))
        pspool = ctx.enter_context(tc.tile_pool(name="ps", bufs=1, space="PSUM"))

        # --- HAM warm-up: junk matmuls on a memset tile (no DMA dep) ------
        warm = wpool.tile([128, 64], F16, tag="warm")
        nc.vector.memset(warm[:], 0.001953125)
        psw = pspool.tile([64, 64], F32, tag="W", bufs=1)
        for _ in range(N_WARM):
            nc.tensor.matmul(psw[:], warm[:], warm[:], start=True, stop=True)

        # --- DMA emission order = consumption order -----------------------
        bt = [None] * n_blocks
        bt[0] = apool.tile([128, K1 * R], F16, tag="bt", bufs=2, name="bt0")
        nc.sync.dma_start(bt[0][:], btp_d.ap()[0:128, :])

        w1 = [None] * NMB

        def emit_w1(mb):
            t = wpool.tile([128, K1 * 512], F16, tag=f"w1_{mb}")
            nc.sync.dma_start(t[:], w1p_d.ap()[mb * 128:(mb + 1) * 128, :])
            w1[mb] = t

        emit_w1(0)

        ones = wpool.tile([128, 1], F16, tag="ones")
        nc.sync.dma_start(ones[:], ones_d.ap()[:])
        b1t = wpool.tile([128, M1], F32, tag="b1t")
        nc.sync.dma_start(b1t[:], b1_d.ap()[:])
        b2t = wpool.tile([128, H], F32, tag="b2t")
        nc.sync.dma_start(b2t[:], b2_d.ap()[:])

        w2 = [None] * H

        def emit_w2(h):
            t = wpool.tile([128, M1 * 128], F16, tag=f"w2_{h}")
            nc.sync.dma_start(t[:], w2p_d.ap()[h * 128:(h + 1) * 128, :])
            w2[h] = t

        for mb in range(1, 4):
            emit_w1(mb)
        emit_w2(0)
        for mb in range(4, 6):
            emit_w1(mb)
        emit_w2(1)
        for mb in range(6, NMB):
            emit_w1(mb)
        for h in range(2, H):
            emit_w2(h)

        sig = wpool.tile([1, NC_ROWS], F32, tag="sig")

        accF = [None] * n_blocks       # final fp16 dot accumulator per block

        def emit_tail(b):
            # partition-reduce 128 -> 1, sigmoid, and the block's output DMA.
            # For the split last block, sigmoid + output DMA go out per half
            # so the first half's HBM-write receipt hides under the second
            # half's reduce/sigmoid.
            psS = pspool.tile([1, R], F32, tag="S", bufs=1)
            if isinstance(accF[b], tuple):          # last block: split halves
                for half, a in enumerate(accF[b]):
                    c0, c1 = half * R // 2, (half + 1) * R // 2
                    nc.tensor.matmul(psS[:, c0:c1], ones[:], a[:], start=True,
                                     stop=True)
                    nc.scalar.activation(
                        sig[0:1, b * R + c0:b * R + c1], psS[:, c0:c1],
                        mybir.ActivationFunctionType.Sigmoid,
                    )
                    nc.sync.dma_start(out_d.ap()[0:1, b * R + c0:b * R + c1],
                                      sig[0:1, b * R + c0:b * R + c1])
                return
            nc.tensor.matmul(psS[:], ones[:], accF[b][:], start=True, stop=True)
            nc.scalar.activation(
                sig[0:1, b * R:(b + 1) * R], psS[:],
                mybir.ActivationFunctionType.Sigmoid,
            )
            nc.sync.dma_start(out_d.ap()[0:1, b * R:(b + 1) * R],
                              sig[0:1, b * R:(b + 1) * R])

        for b in range(n_blocks):
            # prefetch next block's batchT (queued behind the weight bulk)
            if b + 1 < n_blocks:
                t = apool.tile([128, K1 * R], F16, tag="bt", bufs=2)
                nc.sync.dma_start(t[:], btp_d.ap()[(b + 1) * 128:(b + 2) * 128, :])
                bt[b + 1] = t

            # ---- phase 1: innerT[m] = tanh(W1T.T @ batchT + b1) ----
            it = []
            for m in range(M1):
                mb, mo = divmod(m, MB)
                ps = pspool.tile([128, R], F32, tag="p1", bufs=2)
                for k in range(K1):
                    nc.tensor.matmul(
                        ps[:],
                        w1[mb][:, k * 512 + mo * 128:k * 512 + (mo + 1) * 128],
                        bt[b][:, k * R:(k + 1) * R],
                        start=(k == 0), stop=(k == K1 - 1),
                    )
                t = apool.tile([128, R], F16, tag="it", bufs=32)
                nc.scalar.activation(
                    t[:], ps[:], mybir.ActivationFunctionType.Tanh,
                    bias=b1t[:, m:m + 1],
                )
                it.append(t)

            # deferred tail of the previous block: by now its DVE chain is
            # long done, so the reduce matmul costs PE no stall.
            if b > 0:
                emit_tail(b - 1)

            # ---- phase 2 + row-dot, per d_model chunk h ----
            last_blk = b == n_blocks - 1
            acc = None
            for h in range(H):
                if not (last_blk and h == H - 1):
                    ps2 = pspool.tile([128, R], F32, tag="p2", bufs=2)
                    for m in range(M1):
                        nc.tensor.matmul(
                            ps2[:], w2[h][:, m * 128:(m + 1) * 128], it[m][:],
                            start=(m == 0), stop=(m == M1 - 1),
                        )
                    wx = wpool.tile([128, R], F16, tag="wx", bufs=2)
                    nc.scalar.activation(
                        wx[:], ps2[:], mybir.ActivationFunctionType.Tanh,
                        bias=b2t[:, h:h + 1],
                    )
                    final = h == H - 1
                    if h == 0:
                        acc = wpool.tile([128, R], F32, tag="acc", bufs=4,
                                         name="acc0")
                        nc.vector.scalar_tensor_tensor(
                            out=acc[:], in0=wx[:], scalar=1.0,
                            in1=bt[b][:, h * R:(h + 1) * R],
                            op0=mybir.AluOpType.mult, op1=mybir.AluOpType.mult,
                        )
                    else:
                        p = wpool.tile([128, R], F32, tag="p", bufs=2, name="p")
                        nc.vector.scalar_tensor_tensor(
                            out=p[:], in0=wx[:], scalar=1.0,
                            in1=bt[b][:, h * R:(h + 1) * R],
                            op0=mybir.AluOpType.mult, op1=mybir.AluOpType.mult,
                        )
                        nacc = wpool.tile(
                            [128, R], F16 if final else F32,
                            tag="acc16" if final else "acc",
                            bufs=2 if final else 4, name="accn",
                        )
                        nc.vector.scalar_tensor_tensor(
                            out=nacc[:], in0=acc[:], scalar=1.0, in1=p[:],
                            op0=mybir.AluOpType.mult, op1=mybir.AluOpType.add,
                        )
                        acc = nacc
                else:
                    # last h of the last block in two column halves so most
                    # of the ACT/DVE/reduce tail overlaps the second half's
                    # matmuls instead of trailing the whole kernel.
                    halves = []
                    for half in range(2):
                        c0, c1 = half * R // 2, (half + 1) * R // 2
                        psh = pspool.tile([128, R // 2], F32,
                                          tag=f"h7{half}", bufs=1)
                        for m in range(M1):
                            nc.tensor.matmul(
                                psh[:], w2[h][:, m * 128:(m + 1) * 128],
                                it[m][:, c0:c1],
                                start=(m == 0), stop=(m == M1 - 1),
                            )
                        wxh = wpool.tile([128, R // 2], F16, tag="wxh", bufs=2)
                        nc.scalar.activation(
                            wxh[:], psh[:], mybir.ActivationFunctionType.Tanh,
                            bias=b2t[:, h:h + 1],
                        )
                        ph = wpool.tile([128, R // 2], F32, tag="ph", bufs=2, name="ph")
                        nc.vector.scalar_tensor_tensor(
                            out=ph[:], in0=wxh[:], scalar=1.0,
                            in1=bt[b][:, h * R + c0:h * R + c1],
                            op0=mybir.AluOpType.mult, op1=mybir.AluOpType.mult,
                        )
                        a16 = wpool.tile([128, R // 2], F16, tag="acc16h",
                                         bufs=2, name="a16")
                        nc.vector.scalar_tensor_tensor(
                            out=a16[:], in0=acc[:, c0:c1], scalar=1.0, in1=ph[:],
                            op0=mybir.AluOpType.mult, op1=mybir.AluOpType.add,
                        )
                        halves.append(a16)
                    acc = tuple(halves)
            accF[b] = acc

        emit_tail(n_blocks - 1)

    return nc


_CACHED = {}


def _get_nc(n_blocks=N_BLOCKS):
    if n_blocks not in _CACHED:
        _CACHED[n_blocks] = build_bass(n_blocks)
    return _CACHED[n_blocks]


def _prep_in_maps(batch, W1, b1, W2, b2):
    batch = np.ascontiguousarray(batch, dtype=np.float32)
    w1t = np.asarray(W1, dtype=np.float16).T                # [1024, 4096]
    # [k, p, mb, cc] -> [mb, p, k, cc]
    w1p = np.ascontiguousarray(
        w1t.reshape(K1, 128, NMB, 512).transpose(2, 1, 0, 3)
        .reshape(NMB * 128, K1 * 512)
    )
    w2t = np.asarray(W2, dtype=np.float16).T                # [4096, 1024]
    # [m, p, h, c] -> [h, p, m, c]
    w2p = np.ascontiguousarray(
        w2t.reshape(M1, 128, H, 128).transpose(2, 1, 0, 3)
        .reshape(H * 128, M1 * 128)
    )
    b1c = np.ascontiguousarray(np.asarray(b1, dtype=np.float32).reshape(M1, 128).T)
    b2c = np.ascontiguousarray(np.asarray(b2, dtype=np.float32).reshape(H, 128).T)
    ones = np.ones((128, 1), dtype=np.float16)
    batcht = np.ascontiguousarray(batch.T.astype(np.float16))  # [1024, 16384]

    in_maps = []
    for c in range(N_CORES):
        r0, r1 = c * NC_ROWS, (c + 1) * NC_ROWS
        # [k, p, b, r] -> [b, p, k, r]
        btp = np.ascontiguousarray(
            batcht[:, r0:r1].reshape(K1, 128, N_BLOCKS, R).transpose(2, 1, 0, 3)
            .reshape(N_BLOCKS * 128, K1 * R)
        )
        in_maps.append({
            "w1p": w1p,
            "w2p": w2p,
            "b1c": b1c,
            "b2c": b2c,
            "ones": ones,
            "btp": btp,
        })
    return in_maps


def kernel(batch, W1, b1, W2, b2, _trace=False, _trace_kwargs=None):
    in_maps = _prep_in_maps(batch, W1, b1, W2, b2)
    nc = _get_nc()
    res = bass_utils.run_bass_kernel_spmd(
        nc, in_maps, core_ids=list(range(N_CORES)),
        trace=_trace, **(_trace_kwargs or {}),
    )
    out = np.concatenate([res.results[c]["out"][0] for c in range(N_CORES)])
    if _trace:
        return out, res
    return out


# revision 17
# speedup vs baseline: 1.0039x; 1.0039x over previous
"""Trainium2 kernel for nn_MiddleHeadLayer: 2-layer tanh MLP + row-dot + sigmoid.

    inner = tanh(batch @ W1.T + b1)        batch [N, 1024], W1 [4096, 1024]
    wx    = tanh(inner @ W2.T + b2)        W2 [1024, 4096]
    out   = sigmoid(sum(wx * batch, -1))   [N]

Data-parallel over 8 NeuronCores: each core handles N/8 = 2048 rows;
weights replicated, resident in SBUF as fp16 (fp16 matmuls run at full PE
rate, 1 moving column/cycle; absmax error stays ~4e-3).

Per-core dataflow, in blocks of R=512 rows:
  phase 1: innerT[dff, rows] = tanh(W1T.T @ batchT + b1) — stationary W1T
           chunks [128,128], moving batchT [128, 512], fp16 in / f32 PSUM,
           ACT applies the per-partition (d_ff) bias and writes fp16.
  phase 2: wxT[dmodel, rows] = tanh(W2 @ inner.T + b2) — stationary W2T
           chunks [128 dff, 128 dmodel], moving innerT [128, 512]. Output
           partitions are d_model, so b2 is a per-partition ACT bias.
  dot:     acc[dm_chunk, rows] = sum_h wxT_h * batchT_h on DVE (reusing the
           fp16 batchT tiles phase 1 streams); final add writes fp16 so the
           [128,1]-ones partition-reduce matmul runs in one PE pass;
           sigmoid on ACT; one contiguous 2KB output DMA per block.

DMA strategy: the Sync sequencer issues one DMA every ~580ns, so DMA
COUNT (not just bytes) is the startup constraint. All weights and
activations are host-packed so each logical group is ONE contiguous
[128, 4096] 1MB DMA (~78% DMA efficiency): 8 for W1, 8 for W2, 4 for
batchT. The 128-partition slab meaning of each 512-column span differs
(it encodes the contraction chunk), which the matmul APs slice out.
A memset-fed burst of junk matmuls warms the PE clock gate (HAM,
1.2 -> 2.4 GHz) while the first DMAs are in flight.
"""

from contextlib import ExitStack

import numpy as np
import orjson

import concourse.bass as bass
import concourse.tile as tile
from concourse import mybir
from concourse import bass_utils

D_MODEL = 1024
D_FF = 4096
N_TOTAL = 16384
N_CORES = 8
NC_ROWS = N_TOTAL // N_CORES          # 2048 rows per core
R = 512                               # row-block size
N_BLOCKS = NC_ROWS // R               # 4
K1 = D_MODEL // 128                   # 8 contraction chunks for matmul1
M1 = D_FF // 128                      # 32 d_ff chunks
MB = 4                                # m-chunks per w1 column block
NMB = M1 // MB                        # 8 w1 column blocks
H = D_MODEL // 128                    # 8 d_model chunks in phase 2
F16 = mybir.dt.float16
F32 = mybir.dt.float32
N_WARM = 140                          # HAM warm-up matmuls during DMA wait


# ---------------------------------------------------------------------------
# This walrus build rejects >2 sem waits on a single instruction, while Tile's
# wait assignment freely attaches more (e.g. the exit drain gets one wait per
# outstanding logical proc). Legalize at the BIR-JSON level: hoist excess
# waits onto EventSemaphore instructions inserted directly before the
# offending instruction on the same engine stream (identical semantics).
MAX_WAITS = 1
ESEM_WAITS = 2  # EventSemaphore instructions themselves may carry 2 waits


def _legalize_sync_waits(bir: dict) -> dict:
    ctr = 0
    for fn in bir.get("functions", []):
        for blk in fn.get("blocks", []):
            insts = blk.get("instructions")
            if not insts:
                continue
            out = []
            changed = False
            for inst in insts:
                si = inst.get("sync_info")
                ow = (si or {}).get("on_wait") or []
                limit = ESEM_WAITS if inst.get("opcode") == "EventSemaphore" else MAX_WAITS
                if len(ow) > limit:
                    changed = True
                    excess, keep = ow[:-limit], ow[-limit:]
                    for i in range(0, len(excess), ESEM_WAITS):
                        ctr += 1
                        out.append({
                            "debug": inst.get("debug"),
                            "engine": inst["engine"],
                            "ins": [],
                            "outs": [],
                            "name": f"legalwait-{ctr}",
                            "opcode": "EventSemaphore",
                            "sync_info": {
                                "on_update": [],
                                "on_wait": excess[i:i + ESEM_WAITS],
                            },
                        })
                    si["on_wait"] = keep
                out.append(inst)
            if changed:
                blk["instructions"] = out
    return bir


_orig_to_json_bytes = bass.Bass.to_json_bytes


def _patched_to_json_bytes(self) -> bytes:
    return orjson.dumps(_legalize_sync_waits(orjson.loads(_orig_to_json_bytes(self))))


bass.Bass.to_json_bytes = _patched_to_json_bytes


def build_bass(n_blocks=N_BLOCKS):
    nc = bass.Bass("TRN2", target_bir_lowering=False, debug=False)

    # w1p row-block mb: [128, 4096], cols k*512 + mo*128 + c hold
    #   W1T[k*128+p, mb*512 + mo*128 + c]
    w1p_d = nc.dram_tensor("w1p", [NMB * 128, K1 * 512], F16, kind="ExternalInput")
    # w2p row-block h: [128, 4096], cols m*128 + c hold W2T[m*128+p, h*128+c]
    w2p_d = nc.dram_tensor("w2p", [H * 128, M1 * 128], F16, kind="ExternalInput")
    b1_d = nc.dram_tensor("b1c", [128, M1], F32, kind="ExternalInput")
    b2_d = nc.dram_tensor("b2c", [128, H], F32, kind="ExternalInput")
    ones_d = nc.dram_tensor("ones", [128, 1], F16, kind="ExternalInput")
    # btp row-block b: [128, 4096], cols k*512 + r hold batch[b*512+r, k*128+p]
    btp_d = nc.dram_tensor("btp", [N_BLOCKS * 128, K1 * R], F16, kind="ExternalInput")
    out_d = nc.dram_tensor("out", [1, NC_ROWS], F32, kind="ExternalOutput")

    with tile.TileContext(nc) as tc, ExitStack() as ctx:
        wpool = ctx.enter_context(tc.tile_pool(name="weights", bufs=1))
        apool = ctx.enter_context(tc.tile_pool(name="acts", bufs=2))
        pspool = ctx.enter_context(tc.tile_pool(name="ps", bufs=1, space="PSUM"))

        # --- HAM warm-up: junk matmuls on a memset tile (no DMA dep) ------
        warm = wpool.tile([128, 64], F16, tag="warm")
        nc.vector.memset(warm[:], 0.001953125)
        psw = pspool.tile([64, 64], F32, tag="W", bufs=1)
        for _ in range(N_WARM):
            nc.tensor.matmul(psw[:], warm[:], warm[:], start=True, stop=True)

        # --- DMA emission order = consumption order -----------------------
        bt = [None] * n_blocks
        bt[0] = apool.tile([128, K1 * R], F16, tag="bt", bufs=2, name="bt0")
        nc.sync.dma_start(bt[0][:], btp_d.ap()[0:128, :])

        w1 = [None] * NMB

        def emit_w1(mb):
            t = wpool.tile([128, K1 * 512], F16, tag=f"w1_{mb}")
            nc.sync.dma_start(t[:], w1p_d.ap()[mb * 128:(mb + 1) * 128, :])
            w1[mb] = t

        emit_w1(0)

        ones = wpool.tile([128, 1], F16, tag="ones")
        nc.sync.dma_start(ones[:], ones_d.ap()[:])
        b1t = wpool.tile([128, M1], F32, tag="b1t")
        nc.sync.dma_start(b1t[:], b1_d.ap()[:])
        b2t = wpool.tile([128, H], F32, tag="b2t")
        nc.sync.dma_start(b2t[:], b2_d.ap()[:])

        w2 = [None] * H

        def emit_w2(h):
            t = wpool.tile([128, M1 * 128], F16, tag=f"w2_{h}")
            nc.sync.dma_start(t[:], w2p_d.ap()[h * 128:(h + 1) * 128, :])
            w2[h] = t

        for mb in range(1, 4):
            emit_w1(mb)
        emit_w2(0)
        for mb in range(4, 6):
            emit_w1(mb)
        emit_w2(1)
        for mb in range(6, NMB):
            emit_w1(mb)
        for h in range(2, H):
            emit_w2(h)

        sig = wpool.tile([1, NC_ROWS], F32, tag="sig")

        accF = [None] * n_blocks       # final fp16 dot accumulator per block

        def emit_tail(b):
            # partition-reduce 128 -> 1, sigmoid, and the block's output DMA.
            # For the split last block, sigmoid + output DMA go out per half
            # so the first half's HBM-write receipt hides under the second
            # half's reduce/sigmoid.
            psS = pspool.tile([1, R], F32, tag="S", bufs=1)
            if isinstance(accF[b], tuple):          # last block: split halves
                for half, a in enumerate(accF[b]):
                    c0, c1 = half * R // 2, (half + 1) * R // 2
                    nc.tensor.matmul(psS[:, c0:c1], ones[:], a[:], start=True,
                                     stop=True)
                    nc.scalar.activation(
                        sig[0:1, b * R + c0:b * R + c1], psS[:, c0:c1],
                        mybir.ActivationFunctionType.Sigmoid,
                    )
                    nc.sync.dma_start(out_d.ap()[0:1, b * R + c0:b * R + c1],
                                      sig[0:1, b * R + c0:b * R + c1])
                return
            nc.tensor.matmul(psS[:], ones[:], accF[b][:], start=True, stop=True)
            nc.scalar.activation(
                sig[0:1, b * R:(b + 1) * R], psS[:],
                mybir.ActivationFunctionType.Sigmoid,
            )
            nc.sync.dma_start(out_d.ap()[0:1, b * R:(b + 1) * R],
                              sig[0:1, b * R:(b + 1) * R])

        for b in range(n_blocks):
            # prefetch next block's batchT (queued behind the weight bulk)
            if b + 1 < n_blocks:
                t = apool.tile([128, K1 * R], F16, tag="bt", bufs=2)
                nc.sync.dma_start(t[:], btp_d.ap()[(b + 1) * 128:(b + 2) * 128, :])
                bt[b + 1] = t

            # ---- phase 1: innerT[m] = tanh(W1T.T @ batchT + b1) ----
            it = []
            for m in range(M1):
                mb, mo = divmod(m, MB)
                ps = pspool.tile([128, R], F32, tag="p1", bufs=2)
                for k in range(K1):
                    nc.tensor.matmul(
                        ps[:],
                        w1[mb][:, k * 512 + mo * 128:k * 512 + (mo + 1) * 128],
                        bt[b][:, k * R:(k + 1) * R],
                        start=(k == 0), stop=(k == K1 - 1),
                    )
                t = apool.tile([128, R], F16, tag="it", bufs=32)
                nc.scalar.activation(
                    t[:], ps[:], mybir.ActivationFunctionType.Tanh,
                    bias=b1t[:, m:m + 1],
                )
                it.append(t)

            # deferred tail of the previous block: by now its DVE chain is
            # long done, so the reduce matmul costs PE no stall.
            if b > 0:
                emit_tail(b - 1)

            # ---- phase 2 + row-dot, per d_model chunk h ----
            last_blk = b == n_blocks - 1
            acc = None
            for h in range(H):
                if not (last_blk and h == H - 1):
                    ps2 = pspool.tile([128, R], F32, tag="p2", bufs=2)
                    for m in range(M1):
                        nc.tensor.matmul(
                            ps2[:], w2[h][:, m * 128:(m + 1) * 128], it[m][:],
                            start=(m == 0), stop=(m == M1 - 1),
                        )
                    wx = wpool.tile([128, R], F16, tag="wx", bufs=2)
                    nc.scalar.activation(
                        wx[:], ps2[:], mybir.ActivationFunctionType.Tanh,
                        bias=b2t[:, h:h + 1],
                    )
                    final = h == H - 1
                    if h == 0:
                        acc = wpool.tile([128, R], F32, tag="acc", bufs=4,
                                         name="acc0")
                        nc.vector.scalar_tensor_tensor(
                            out=acc[:], in0=wx[:], scalar=1.0,
                            in1=bt[b][:, h * R:(h + 1) * R],
                            op0=mybir.AluOpType.mult, op1=mybir.AluOpType.mult,
                        )
                    else:
                        p = wpool.tile([128, R], F32, tag="p", bufs=2, name="p")
                        nc.vector.scalar_tensor_tensor(
                            out=p[:], in0=wx[:], scalar=1.0,
                            in1=bt[b][:, h * R:(h + 1) * R],
                            op0=mybir.AluOpType.mult, op1=mybir.AluOpType.mult,
                        )
                        nacc = wpool.tile(
                            [128, R], F16 if final else F32,
                            tag="acc16" if final else "acc",
                            bufs=2 if final else 4, name="accn",
                        )
                        nc.vector.scalar_tensor_tensor(
                            out=nacc[:], in0=acc[:], scalar=1.0, in1=p[:],
                            op0=mybir.AluOpType.mult, op1=mybir.AluOpType.add,
                        )
                        acc = nacc
                else:
                    # last h of the last block in two column halves so most
                    # of the ACT/DVE/reduce tail overlaps the second half's
                    # matmuls instead of trailing the whole kernel.
                    halves = []
                    for half in range(2):
                        c0, c1 = half * R // 2, (half + 1) * R // 2
                        psh = pspool.tile([128, R // 2], F32,
                                          tag=f"h7{half}", bufs=1)
                        for m in range(M1):
                            nc.tensor.matmul(
                                psh[:], w2[h][:, m * 128:(m + 1) * 128],
                                it[m][:, c0:c1],
                                start=(m == 0), stop=(m == M1 - 1),
                            )
                        wxh = wpool.tile([128, R // 2], F16, tag="wxh", bufs=2)
                        nc.scalar.activation(
                            wxh[:], psh[:], mybir.ActivationFunctionType.Tanh,
                            bias=b2t[:, h:h + 1],
                        )
                        ph = wpool.tile([128, R // 2], F32, tag="ph", bufs=2, name="ph")
                        nc.vector.scalar_tensor_tensor(
                            out=ph[:], in0=wxh[:], scalar=1.0,
                            in1=bt[b][:, h * R + c0:h * R + c1],
                            op0=mybir.AluOpType.mult, op1=mybir.AluOpType.mult,
                        )
                        a16 = wpool.tile([128, R // 2], F16, tag="acc16h",
                                         bufs=2, name="a16")
                        nc.vector.scalar_tensor_tensor(
                            out=a16[:], in0=acc[:, c0:c1], scalar=1.0, in1=ph[:],
                            op0=mybir.AluOpType.mult, op1=mybir.AluOpType.add,
                        )
                        halves.append(a16)
                    acc = tuple(halves)
            accF[b] = acc

        emit_tail(n_blocks - 1)

    return nc


_CACHED = {}


def _get_nc(n_blocks=N_BLOCKS):
    if n_blocks not in _CACHED:
        _CACHED[n_blocks] = build_bass(n_blocks)
    return _CACHED[n_blocks]


def _prep_in_maps(batch, W1, b1, W2, b2):
    batch = np.ascontiguousarray(batch, dtype=np.float32)
    w1t = np.asarray(W1, dtype=np.float16).T                # [1024, 4096]
    # [k, p, mb, cc] -> [mb, p, k, cc]
    w1p = np.ascontiguousarray(
        w1t.reshape(K1, 128, NMB, 512).transpose(2, 1, 0, 3)
        .reshape(NMB * 128, K1 * 512)
    )
    w2t = np.asarray(W2, dtype=np.float16).T                # [4096, 1024]
    # [m, p, h, c] -> [h, p, m, c]
    w2p = np.ascontiguousarray(
        w2t.reshape(M1, 128, H, 128).transpose(2, 1, 0, 3)
        .reshape(H * 128, M1 * 128)
    )
    b1c = np.ascontiguousarray(np.asarray(b1, dtype=np.float32).reshape(M1, 128).T)
    b2c = np.ascontiguousarray(np.asarray(b2, dtype=np.float32).reshape(H, 128).T)
    ones = np.ones((128, 1), dtype=np.float16)
    batcht = np.ascontiguousarray(batch.T.astype(np.float16))  # [1024, 16384]

    in_maps = []
    for c in range(N_CORES):
        r0, r1 = c * NC_ROWS, (c + 1) * NC_ROWS
        # [k, p, b, r] -> [b, p, k, r]
        btp = np.ascontiguousarray(
            batcht[:, r0:r1].reshape(K1, 128, N_BLOCKS, R).transpose(2, 1, 0, 3)
            .reshape(N_BLOCKS * 128, K1 * R)
        )
        in_maps.append({
            "w1p": w1p,
            "w2p": w2p,
            "b1c": b1c,
            "b2c": b2c,
            "ones": ones,
            "btp": btp,
        })
    return in_maps


def kernel(batch, W1, b1, W2, b2, _trace=False, _trace_kwargs=None):
    in_maps = _prep_in_maps(batch, W1, b1, W2, b2)
    nc = _get_nc()
    res = bass_utils.run_bass_kernel_spmd(
        nc, in_maps, core_ids=list(range(N_CORES)),
        trace=_trace, **(_trace_kwargs or {}),
    )
    out = np.concatenate([res.results[c]["out"][0] for c in range(N_CORES)])
    if _trace:
        return out, res
    return out
